# revision 1
# baseline (speedup 1.0000x reference)
"""Trainium2 Bass kernel for ConvMessageAggregator.

Computes, for each node n (messages: [N, 16, 688] fp32):
  f1[i] = relu(w10*x[i] + w11*x[i+2] + b1)      i in 0..13   (dilated 2-tap conv)
  f2[i] = relu(w20*f1[i] + w21*f1[i+2] + b2)    i in 0..11
  out   = relu(sum_k mlp_w[k] * f2[6+k] + mlp_b)             -> [N, 688]

Only f2 rows 6..11 are consumed, which depend on f1 rows 6..13, which depend
on x rows 6..15 -- so the kernel only reads the last 10 (contiguous) rows of
each node's 16-row block (10/16 of the input bytes).

Sharding: pure data parallel; node axis split across 8 NeuronCores, all
conv/MLP scalars baked into the instruction stream as immediates at trace
time (the program is rebuilt per call, so arbitrary weights are handled).

Per-core pipeline (2048 nodes = 16 tiles of 128 nodes on partitions):
  DMA  x[128, 10, 688]                                (HWDGE, one 3.5MB DMA)
  DVE  u1 = (x_other * r1) + x_pivot    [128, 8, 688] (scalar_tensor_tensor)
  ACT  f1 = Relu(p1*u1 + b1)            [128, 8, 688]
  DVE  u2 = (f1_other * r2) + f1_pivot  [128, 6, 688]
  ACT  f2 = Relu(p2*u2 + b2)            [128, 6, 688]
  DVE  5x binary-combine tree over the 6 rows (weight ratios all <= 1)
  ACT  out = Relu(w_anchor*t + mlp_b)   [128, 688]
  DMA  out tile -> DRAM
"""

import sys

for _p in ("/opt/trn_rl_repo",):
    if _p not in sys.path:
        sys.path.insert(0, _p)

import numpy as np

import concourse.bass as bass
import concourse.tile as tile
from concourse import mybir
from concourse.bass_utils import run_bass_kernel_spmd

N_FULL, L, MSG = 16384, 16, 688
N_CORES = 8
N_LOCAL = N_FULL // N_CORES  # 2048
P = 128                      # nodes per tile (partition dim)
NTILES = N_LOCAL // P        # 16
R0, NROWS = 6, 10            # input rows actually used: 6..15 (contiguous)

F32 = mybir.dt.float32
AF = mybir.ActivationFunctionType
OP = mybir.AluOpType


def _split_multi_waits(nc):
    """TPB instructions encode at most ONE semaphore wait; this walrus build's
    codegen rejects instructions with more. Hoist extra waits into standalone
    EventSemaphore ops on the same (in-order) sequencer -- semantically
    identical to the attached wait."""
    for func in nc.m.functions:
        for bb in func.blocks:
            insts = list(bb.instructions)
            if not any(
                i.sync_info is not None and len(i.sync_info.on_wait) > 1
                for i in insts
            ):
                continue
            new = []
            for inst in insts:
                si = inst.sync_info
                if si is not None and len(si.on_wait) > 1:
                    waits = list(si.on_wait)
                    for j, w in enumerate(waits[:-1]):
                        new.append(
                            mybir.InstEventSemaphore(
                                name=f"{inst.name}-hoistw{j}",
                                engine=inst.engine,
                                sync_info=mybir.SyncInfo(on_wait=[w], on_update=[]),
                            )
                        )
                    inst.sync_info = mybir.SyncInfo(
                        on_wait=[waits[-1]], on_update=list(si.on_update)
                    )
                new.append(inst)
            bb.instructions = new


def _conv_split(wa, wb):
    """Factor pre[i] = wa*in[i] + wb*in[i+2] as pivot*(in[pv] + r*in[ot]).

    Returns (pivot_weight, ratio, pivot_row_off, other_row_off) with |ratio|<=1.
    """
    if abs(wa) >= abs(wb):
        return wa, (wb / wa if wa != 0.0 else 0.0), 0, 2
    return wb, wa / wb, 2, 0


def build_program(w10, w11, b1, w20, w21, b2, mlp_w, mlp_b):
    nc = bass.Bass(trn_type="TRN2", name="conv_msg_agg")
    x = nc.dram_tensor("x", [N_LOCAL, L, MSG], F32, kind="ExternalInput")
    out = nc.dram_tensor("out", [N_LOCAL, MSG], F32, kind="ExternalOutput")

    p1, r1, pv1, ot1 = _conv_split(w10, w11)
    p2, r2, pv2, ot2 = _conv_split(w20, w21)

    # mlp weighted-sum plan: anchor a = argmax |mlp_w|.  For each nonzero k,
    # G[k] = s_k * relu(conv2[k]) with s_k = |mlp_w[k]/mlp_w[a]| <= 1, computed
    # in ONE ACT op from u2 (relu(s_k*p2*u2[k] + s_k*b2)).  Then
    # sum = mlp_w[a] * sum_k tau_k G[k] (tau_k = sign) via plain add/sub
    # tensor_tensor ops on the otherwise-idle GPSIMD engine (walrus rejects
    # TensorScalarPtr on Pool, so the tree must be scalar-free).
    nzk = [k for k in range(6) if mlp_w[k] != 0.0]
    anchor = max(nzk, key=lambda k: abs(mlp_w[k])) if nzk else -1
    wa = mlp_w[anchor] if nzk else 0.0

    with tile.TileContext(nc) as tc:
        with (
            tc.tile_pool(name="bias", bufs=1) as pool_b,
            tc.tile_pool(name="xin", bufs=2) as pool_x,
            tc.tile_pool(name="work", bufs=2) as pool_w,
            tc.tile_pool(name="gbuf", bufs=2) as pool_g,
            tc.tile_pool(name="outp", bufs=2) as pool_o,
        ):
            # activation() needs SBUF [P,1] bias vectors for non-Copy funcs
            b1c = pool_b.tile([P, 1], F32, tag="b1")
            nc.vector.memset(b1c[:], b1)
            gbias = {}
            for k in nzk:
                s_k = abs(mlp_w[k] / wa)
                gbias[k] = pool_b.tile([P, 1], F32, tag=f"gb{k}", name=f"gb{k}")
                nc.vector.memset(gbias[k][:], s_k * b2)
            mbc = pool_b.tile([P, 1], F32, tag="mb")
            nc.vector.memset(mbc[:], mlp_b)

            for it in range(NTILES):
                n0 = it * P
                xt = pool_x.tile([P, NROWS, MSG], F32, tag="x")
                nc.gpsimd.dma_start(out=xt[:], in_=x[n0 : n0 + P, R0 : R0 + NROWS, :])

                # conv1: u1 = x_pv + r1*x_ot (DVE), then relu-affine IN PLACE
                # (ACT) -- saves 22KB/partition so the chain double-buffers
                u1 = pool_w.tile([P, 8, MSG], F32, tag="u1")
                if p1 == 0.0:
                    nc.vector.memset(u1[:], max(b1, 0.0))
                else:
                    nc.vector.scalar_tensor_tensor(
                        out=u1[:],
                        in0=xt[:, ot1 : ot1 + 8, :],
                        scalar=r1,
                        in1=xt[:, pv1 : pv1 + 8, :],
                        op0=OP.mult,
                        op1=OP.add,
                    )
                    nc.scalar.activation(
                        out=u1[:], in_=u1[:], func=AF.Relu, bias=b1c[:], scale=p1
                    )

                # conv2 pre-activation (DVE)
                u2 = pool_w.tile([P, 6, MSG], F32, tag="u2")
                if p2 == 0.0:
                    nc.vector.memset(u2[:], 0.0)
                    u2_scale, u2_bias_val = 0.0, b2
                else:
                    nc.vector.scalar_tensor_tensor(
                        out=u2[:],
                        in0=u1[:, ot2 : ot2 + 6, :],
                        scalar=r2,
                        in1=u1[:, pv2 : pv2 + 6, :],
                        op0=OP.mult,
                        op1=OP.add,
                    )
                    u2_scale, u2_bias_val = p2, b2

                # G[k] = s_k*relu(conv2[k]) fused into one ACT op per row
                terms = []  # (tau, tile_ap)
                for k in nzk:
                    s_k = abs(mlp_w[k] / wa)
                    gk = pool_g.tile([P, MSG], F32, tag=f"g{k}", name=f"g{k}")
                    nc.scalar.activation(
                        out=gk[:],
                        in_=u2[:, k, :],
                        func=AF.Relu,
                        bias=gbias[k][:],
                        scale=s_k * u2_scale,
                    )
                    terms.append((1 if mlp_w[k] / wa > 0 else -1, gk[:]))

                # scalar-free signed combine tree on GPSIMD, in-place into
                # the left operand's tile
                while len(terms) > 1:
                    nxt = []
                    for i in range(0, len(terms) - 1, 2):
                        ta, aa = terms[i]
                        tb, ab = terms[i + 1]
                        op = OP.add if ta == tb else OP.subtract
                        nc.gpsimd.tensor_tensor(out=aa, in0=aa, in1=ab, op=op)
                        nxt.append((ta, aa))
                    if len(terms) % 2:
                        nxt.append(terms[-1])
                    terms = nxt

                ot = pool_o.tile([P, MSG], F32, tag="o")
                if terms:
                    tau, aa = terms[0]
                    nc.scalar.activation(
                        out=ot[:], in_=aa, func=AF.Relu, bias=mbc[:], scale=wa * tau
                    )
                else:
                    nc.vector.memset(ot[:], max(mlp_b, 0.0))
                nc.gpsimd.dma_start(out=out[n0 : n0 + P, :], in_=ot[:])
    _split_multi_waits(nc)
    return nc


def run(inputs, trace=False, **spmd_kwargs):
    """Build + run on 8 cores. Returns (full_output, BassKernelResults)."""
    msgs = np.asarray(inputs["messages"], dtype=np.float32)
    assert msgs.shape == (N_FULL, L, MSG), msgs.shape
    if not msgs.flags["C_CONTIGUOUS"]:
        msgs = np.ascontiguousarray(msgs)

    c1w = np.asarray(inputs["conv1_w"], dtype=np.float64)
    c2w = np.asarray(inputs["conv2_w"], dtype=np.float64)
    mlw = np.asarray(inputs["mlp_w"], dtype=np.float64)
    nc = build_program(
        float(c1w[0]),
        float(c1w[1]),
        float(np.asarray(inputs["conv1_b"], dtype=np.float64)),
        float(c2w[0]),
        float(c2w[1]),
        float(np.asarray(inputs["conv2_b"], dtype=np.float64)),
        [float(v) for v in mlw],
        float(np.asarray(inputs["mlp_b"], dtype=np.float64)),
    )

    in_maps = [
        {"x": msgs[i * N_LOCAL : (i + 1) * N_LOCAL]} for i in range(N_CORES)
    ]
    res = run_bass_kernel_spmd(
        nc, in_maps, core_ids=list(range(N_CORES)), trace=trace, **spmd_kwargs
    )
    full = np.concatenate([r["out"] for r in res.results], axis=0)
    return full, res


def kernel(**inputs) -> np.ndarray:
    return run(inputs, trace=False)[0]



# revision 4
# speedup vs baseline: 1.3129x; 1.3129x over previous
"""Trainium2 Bass kernel for ConvMessageAggregator.

Computes, for each node n (messages: [N, 16, 688] fp32):
  f1[i] = relu(w10*x[i] + w11*x[i+2] + b1)      i in 0..13   (dilated 2-tap conv)
  f2[i] = relu(w20*f1[i] + w21*f1[i+2] + b2)    i in 0..11
  out   = relu(sum_k mlp_w[k] * f2[6+k] + mlp_b)             -> [N, 688]

Only f2 rows 6..11 are consumed -> f1 rows 6..13 -> x rows 6..15, so the
kernel reads just the last 10 (contiguous) rows of each node (10/16 of the
input bytes).  Sharding: pure data parallel, node axis split across 8 cores;
all scalar params are baked into the instruction stream at trace time.

Per-core pipeline (2048 nodes = 16 tiles of 128 nodes on partitions), fp16
intermediates (inputs are cast fp32->fp16 by the load DMA itself):
  DMA  x[128, 10*688] fp16     (SWDGE cast load, 3.5MB HBM-side)
  DVE  u1 = x_ot*r1 + x_pv     [128, 8*688]  (STT, flat 2D APs, 2x fp16 mode)
  ACT  f1 = Relu(p1*u1 + b1)   in place
  DVE  u2 = f1_ot*r2 + f1_pv   [128, 6*688]
  ACT  f2 = Relu(p2*u2 + b2)   in place
  DVE  5x STT fold (rows sorted by |mlp_w|, all ratios <= 1) in place
  ACT  out = Relu(w_max*t + mlp_b) -> fp32
  DMA  out tile -> DRAM        (HWDGE on sync, no Pool involvement)

vs the previous version: the 6-row weighted combine moved from GPSIMD
(80 small tensor_tensor ops, 241us -> the bottleneck engine at 77% busy)
to 5 STT ops on DVE; fp16 halves DVE streaming cost; one fused ACT per
stage instead of 6 small G ops; stores on HWDGE so Pool only dispatches
the 16 cast loads.
"""

import sys

for _p in ("/opt/trn_rl_repo",):
    if _p not in sys.path:
        sys.path.insert(0, _p)

import numpy as np

import concourse.bass as bass
import concourse.tile as tile
from concourse import mybir
from concourse.bass_utils import run_bass_kernel_spmd

N_FULL, L, MSG = 16384, 16, 688
N_CORES = 8
N_LOCAL = N_FULL // N_CORES  # 2048
P = 128                      # nodes per tile (partition dim)
NTILES = N_LOCAL // P        # 16
R0, NROWS = 6, 10            # input rows actually used: 6..15 (contiguous)

F32 = mybir.dt.float32
F16 = mybir.dt.float16
AF = mybir.ActivationFunctionType
OP = mybir.AluOpType


def _split_multi_waits(nc):
    """TPB instructions encode at most ONE semaphore wait; this walrus build's
    codegen rejects instructions with more. Hoist extra waits into standalone
    EventSemaphore ops on the same (in-order) sequencer -- semantically
    identical to the attached wait."""
    for func in nc.m.functions:
        for bb in func.blocks:
            insts = list(bb.instructions)
            if not any(
                i.sync_info is not None and len(i.sync_info.on_wait) > 1
                for i in insts
            ):
                continue
            new = []
            for inst in insts:
                si = inst.sync_info
                if si is not None and len(si.on_wait) > 1:
                    waits = list(si.on_wait)
                    for j, w in enumerate(waits[:-1]):
                        new.append(
                            mybir.InstEventSemaphore(
                                name=f"{inst.name}-hoistw{j}",
                                engine=inst.engine,
                                sync_info=mybir.SyncInfo(on_wait=[w], on_update=[]),
                            )
                        )
                    inst.sync_info = mybir.SyncInfo(
                        on_wait=[waits[-1]], on_update=list(si.on_update)
                    )
                new.append(inst)
            bb.instructions = new


def _conv_split(wa, wb):
    """Factor pre[i] = wa*in[i] + wb*in[i+2] as pivot*(in[pv] + r*in[ot]).

    Returns (pivot_weight, ratio, pivot_row_off, other_row_off) with |ratio|<=1.
    """
    if abs(wa) >= abs(wb):
        return wa, (wb / wa if wa != 0.0 else 0.0), 0, 2
    return wb, wa / wb, 2, 0


def build_program(w10, w11, b1, w20, w21, b2, mlp_w, mlp_b):
    nc = bass.Bass(trn_type="TRN2", name="conv_msg_agg")
    x = nc.dram_tensor("x", [N_LOCAL, L, MSG], F32, kind="ExternalInput")
    out = nc.dram_tensor("out", [N_LOCAL, MSG], F32, kind="ExternalOutput")

    p1, r1, pv1, ot1 = _conv_split(w10, w11)
    p2, r2, pv2, ot2 = _conv_split(w20, w21)

    # mlp fold plan: rows with nonzero weight, sorted ascending |w|; fold
    # acc -> next row with ratio w_prev/w_next (always |.| <= 1), final
    # scale = largest |w|, applied with mlp_b inside the last ACT op.
    nzk = sorted(
        (k for k in range(6) if mlp_w[k] != 0.0), key=lambda k: abs(mlp_w[k])
    )

    with tile.TileContext(nc) as tc:
        with (
            tc.tile_pool(name="bias", bufs=1) as pool_b,
            tc.tile_pool(name="xin", bufs=4) as pool_x,
            tc.tile_pool(name="wk1", bufs=2) as pool_1,
            tc.tile_pool(name="wk2", bufs=2) as pool_2,
            tc.tile_pool(name="outp", bufs=3) as pool_o,
        ):
            # activation() needs SBUF [P,1] bias vectors for non-Copy funcs
            b1c = pool_b.tile([P, 1], F32, tag="b1")
            nc.vector.memset(b1c[:], b1)
            b2c = pool_b.tile([P, 1], F32, tag="b2")
            nc.vector.memset(b2c[:], b2)
            mbc = pool_b.tile([P, 1], F32, tag="mb")
            nc.vector.memset(mbc[:], mlp_b)

            for it in range(NTILES):
                n0 = it * P
                xt = pool_x.tile([P, NROWS * MSG], F16, tag="x")
                nc.gpsimd.dma_start(
                    out=xt[:],
                    in_=x[n0 : n0 + P, R0 : R0 + NROWS, :].rearrange(
                        "p r m -> p (r m)"
                    ),
                )

                # conv1: u1 = x_pv + r1*x_ot (flat 2D slices), relu-affine
                # in place on ACT
                u1 = pool_1.tile([P, 8 * MSG], F16, tag="u1")
                if p1 == 0.0:
                    nc.vector.memset(u1[:], max(b1, 0.0))
                else:
                    nc.vector.scalar_tensor_tensor(
                        out=u1[:],
                        in0=xt[:, ot1 * MSG : ot1 * MSG + 8 * MSG],
                        scalar=r1,
                        in1=xt[:, pv1 * MSG : pv1 * MSG + 8 * MSG],
                        op0=OP.mult,
                        op1=OP.add,
                    )
                    nc.scalar.activation(
                        out=u1[:], in_=u1[:], func=AF.Relu, bias=b1c[:], scale=p1
                    )

                # conv2 + relu-affine, in place
                u2 = pool_2.tile([P, 6 * MSG], F16, tag="u2")
                if p2 == 0.0:
                    nc.vector.memset(u2[:], max(b2, 0.0))
                else:
                    nc.vector.scalar_tensor_tensor(
                        out=u2[:],
                        in0=u1[:, ot2 * MSG : ot2 * MSG + 6 * MSG],
                        scalar=r2,
                        in1=u1[:, pv2 * MSG : pv2 * MSG + 6 * MSG],
                        op0=OP.mult,
                        op1=OP.add,
                    )
                    nc.scalar.activation(
                        out=u2[:], in_=u2[:], func=AF.Relu, bias=b2c[:], scale=p2
                    )

                # weighted 6-row fold, in place into the larger-|w| row
                def row(k):
                    return u2[:, k * MSG : (k + 1) * MSG]

                for i in range(1, len(nzk)):
                    ka, kb = nzk[i - 1], nzk[i]
                    nc.vector.scalar_tensor_tensor(
                        out=row(kb),
                        in0=row(ka),
                        scalar=mlp_w[ka] / mlp_w[kb],
                        in1=row(kb),
                        op0=OP.mult,
                        op1=OP.add,
                    )

                ot = pool_o.tile([P, MSG], F32, tag="o")
                if nzk:
                    nc.scalar.activation(
                        out=ot[:],
                        in_=row(nzk[-1]),
                        func=AF.Relu,
                        bias=mbc[:],
                        scale=mlp_w[nzk[-1]],
                    )
                else:
                    nc.vector.memset(ot[:], max(mlp_b, 0.0))
                nc.sync.dma_start(out=out[n0 : n0 + P, :], in_=ot[:])
    _split_multi_waits(nc)
    return nc


def run(inputs, trace=False, **spmd_kwargs):
    """Build + run on 8 cores. Returns (full_output, BassKernelResults)."""
    msgs = np.asarray(inputs["messages"], dtype=np.float32)
    assert msgs.shape == (N_FULL, L, MSG), msgs.shape
    if not msgs.flags["C_CONTIGUOUS"]:
        msgs = np.ascontiguousarray(msgs)

    c1w = np.asarray(inputs["conv1_w"], dtype=np.float64)
    c2w = np.asarray(inputs["conv2_w"], dtype=np.float64)
    mlw = np.asarray(inputs["mlp_w"], dtype=np.float64)
    nc = build_program(
        float(c1w[0]),
        float(c1w[1]),
        float(np.asarray(inputs["conv1_b"], dtype=np.float64)),
        float(c2w[0]),
        float(c2w[1]),
        float(np.asarray(inputs["conv2_b"], dtype=np.float64)),
        [float(v) for v in mlw],
        float(np.asarray(inputs["mlp_b"], dtype=np.float64)),
    )

    in_maps = [
        {"x": msgs[i * N_LOCAL : (i + 1) * N_LOCAL]} for i in range(N_CORES)
    ]
    res = run_bass_kernel_spmd(
        nc, in_maps, core_ids=list(range(N_CORES)), trace=trace, **spmd_kwargs
    )
    full = np.concatenate([r["out"] for r in res.results], axis=0)
    return full, res


def kernel(**inputs) -> np.ndarray:
    return run(inputs, trace=False)[0]


# revision 5
# speedup vs baseline: 1.4835x; 1.1300x over previous
"""Trainium2 Bass kernel for ConvMessageAggregator.

Computes, for each node n (messages: [N, 16, 688] fp32):
  f1[i] = relu(w10*x[i] + w11*x[i+2] + b1)      i in 0..13   (dilated 2-tap conv)
  f2[i] = relu(w20*f1[i] + w21*f1[i+2] + b2)    i in 0..11
  out   = relu(sum_k mlp_w[k] * f2[6+k] + mlp_b)             -> [N, 688]

Only f2 rows 6..11 are consumed -> f1 rows 6..13 -> x rows 6..15, so the
kernel reads just the last 10 (contiguous) rows of each node (10/16 of the
input bytes).  Sharding: pure data parallel, node axis split across 8 cores;
all scalar params are baked into the instruction stream at trace time.

Per-core pipeline (2048 nodes = 16 tiles of 128 nodes on partitions).
Inputs stay fp32 through the DMA (a cast-on-DMA load measured 1.56x slower
per SDMA engine than a plain copy); conv1's STT runs at 1x rate regardless
of dtype, so it reads fp32 and WRITES fp16 -- everything downstream runs in
fp16 where DVE gets its 2x/4x perf modes:
  DMA  x[128, 10*688] fp32       (SWDGE load, 3.5MB, flat AP)
  DVE  u1 = (x_ot*r1 + x_pv)     STT fp32->fp16  [128, 8*688]
  ACT  f1 = Relu(p1*u1 + b1)     in place, fp16
  DVE  ys = u1_ot*r2             tensor_scalar, 4x mode
  DVE  u2 = ys + u1_pv           tensor_tensor, 2x mode
  ACT  6x G[k] = Relu(s_k*p2*u2[k] + s_k*b2)  in place (relu homogeneity
       folds |mlp_w[k]/w_anchor| into each row's relu)
  DVE/Pool  +- pairwise tree over the 6 G rows (TT add/sub, 2x mode;
       one level-1 pair runs on the otherwise idle GPSIMD engine)
  ACT  out = Relu(tau*w_anchor*t + mlp_b) -> fp32
  DMA  out tile -> DRAM          (HWDGE on sync, separate ring)

vs the first rewrite (258us): no cast-DMA penalty, f2's big ACT replaced by
the 6 fused G ops, the 5-op STT fold (1x only -- STT has no 2x uop) replaced
by a 2x-mode TT tree.  Engine model per core: DVE ~166us, ACT ~162us,
DMA ~173us, Pool ~60us.
"""

import sys

for _p in ("/opt/trn_rl_repo",):
    if _p not in sys.path:
        sys.path.insert(0, _p)

import numpy as np

import concourse.bass as bass
import concourse.tile as tile
from concourse import mybir
from concourse.bass_utils import run_bass_kernel_spmd

N_FULL, L, MSG = 16384, 16, 688
N_CORES = 8
N_LOCAL = N_FULL // N_CORES  # 2048
P = 128                      # nodes per tile (partition dim)
NTILES = N_LOCAL // P        # 16
R0, NROWS = 6, 10            # input rows actually used: 6..15 (contiguous)

F32 = mybir.dt.float32
F16 = mybir.dt.float16
AF = mybir.ActivationFunctionType
OP = mybir.AluOpType


def _split_multi_waits(nc):
    """TPB instructions encode at most ONE semaphore wait; this walrus build's
    codegen rejects instructions with more. Hoist extra waits into standalone
    EventSemaphore ops on the same (in-order) sequencer -- semantically
    identical to the attached wait."""
    for func in nc.m.functions:
        for bb in func.blocks:
            insts = list(bb.instructions)
            if not any(
                i.sync_info is not None and len(i.sync_info.on_wait) > 1
                for i in insts
            ):
                continue
            new = []
            for inst in insts:
                si = inst.sync_info
                if si is not None and len(si.on_wait) > 1:
                    waits = list(si.on_wait)
                    for j, w in enumerate(waits[:-1]):
                        new.append(
                            mybir.InstEventSemaphore(
                                name=f"{inst.name}-hoistw{j}",
                                engine=inst.engine,
                                sync_info=mybir.SyncInfo(on_wait=[w], on_update=[]),
                            )
                        )
                    inst.sync_info = mybir.SyncInfo(
                        on_wait=[waits[-1]], on_update=list(si.on_update)
                    )
                new.append(inst)
            bb.instructions = new


def _conv_split(wa, wb):
    """Factor pre[i] = wa*in[i] + wb*in[i+2] as pivot*(in[pv] + r*in[ot]).

    Returns (pivot_weight, ratio, pivot_row_off, other_row_off) with |ratio|<=1.
    """
    if abs(wa) >= abs(wb):
        return wa, (wb / wa if wa != 0.0 else 0.0), 0, 2
    return wb, wa / wb, 2, 0


def build_program(w10, w11, b1, w20, w21, b2, mlp_w, mlp_b):
    nc = bass.Bass(trn_type="TRN2", name="conv_msg_agg")
    x = nc.dram_tensor("x", [N_LOCAL, L, MSG], F32, kind="ExternalInput")
    out = nc.dram_tensor("out", [N_LOCAL, MSG], F32, kind="ExternalOutput")

    p1, r1, pv1, ot1 = _conv_split(w10, w11)
    p2, r2, pv2, ot2 = _conv_split(w20, w21)

    # mlp plan: anchor = argmax |w|; G[k] = s_k*relu(p2*u2[k] + b2) with
    # s_k = |w_k/w_anchor| <= 1 folded into the ACT op (relu homogeneity,
    # s_k > 0).  Sum = w_anchor * sum_k tau_k G[k] via a scalar-free +-
    # pairwise tree.
    nzk = [k for k in range(6) if mlp_w[k] != 0.0]
    anchor = max(nzk, key=lambda k: abs(mlp_w[k])) if nzk else -1
    wa = mlp_w[anchor] if nzk else 0.0

    with tile.TileContext(nc) as tc:
        with (
            tc.tile_pool(name="bias", bufs=1) as pool_b,
            tc.tile_pool(name="xin", bufs=4) as pool_x,
            tc.tile_pool(name="wk1", bufs=2) as pool_1,
            tc.tile_pool(name="wks", bufs=2) as pool_s,
            tc.tile_pool(name="wk2", bufs=2) as pool_2,
            tc.tile_pool(name="outp", bufs=3) as pool_o,
        ):
            # activation() needs SBUF [P,1] bias vectors for non-Copy funcs
            b1c = pool_b.tile([P, 1], F32, tag="b1")
            nc.vector.memset(b1c[:], b1)
            gbias = {}
            for k in nzk:
                s_k = abs(mlp_w[k] / wa)
                gbias[k] = pool_b.tile([P, 1], F32, tag=f"gb{k}", name=f"gb{k}")
                nc.vector.memset(gbias[k][:], s_k * b2)
            mbc = pool_b.tile([P, 1], F32, tag="mb")
            nc.vector.memset(mbc[:], mlp_b)

            for it in range(NTILES):
                n0 = it * P
                xt = pool_x.tile([P, NROWS * MSG], F32, tag="x")
                nc.gpsimd.dma_start(
                    out=xt[:],
                    in_=x[n0 : n0 + P, R0 : R0 + NROWS, :].rearrange(
                        "p r m -> p (r m)"
                    ),
                )

                # conv1: u1 = x_pv + r1*x_ot (STT, 1x either way, so it does
                # the fp32->fp16 narrowing for free), relu-affine in place
                u1 = pool_1.tile([P, 8 * MSG], F16, tag="u1")
                if p1 == 0.0:
                    nc.vector.memset(u1[:], max(b1, 0.0))
                else:
                    nc.vector.scalar_tensor_tensor(
                        out=u1[:],
                        in0=xt[:, ot1 * MSG : ot1 * MSG + 8 * MSG],
                        scalar=r1,
                        in1=xt[:, pv1 * MSG : pv1 * MSG + 8 * MSG],
                        op0=OP.mult,
                        op1=OP.add,
                    )
                    nc.scalar.activation(
                        out=u1[:], in_=u1[:], func=AF.Relu, bias=b1c[:], scale=p1
                    )

                # conv2 in fp16: scale at 4x (tensor_scalar) + add at 2x
                # (tensor_tensor) beats one 1x STT
                u2 = pool_2.tile([P, 6 * MSG], F16, tag="u2")
                if p2 == 0.0:
                    nc.vector.memset(u2[:], 0.0)
                    u2_scale = 0.0
                else:
                    ys = pool_s.tile([P, 6 * MSG], F16, tag="ys")
                    nc.vector.tensor_scalar_mul(
                        out=ys[:],
                        in0=u1[:, ot2 * MSG : ot2 * MSG + 6 * MSG],
                        scalar1=r2,
                    )
                    nc.vector.tensor_tensor(
                        out=u2[:],
                        in0=ys[:],
                        in1=u1[:, pv2 * MSG : pv2 * MSG + 6 * MSG],
                        op=OP.add,
                    )
                    u2_scale = p2

                # G[k] = s_k*relu(conv2[k]) fused into one ACT op per row,
                # in place on u2's rows
                def row(k):
                    return u2[:, k * MSG : (k + 1) * MSG]

                terms = []  # (tau, row_idx)
                for k in nzk:
                    s_k = abs(mlp_w[k] / wa)
                    nc.scalar.activation(
                        out=row(k),
                        in_=row(k),
                        func=AF.Relu,
                        bias=gbias[k][:],
                        scale=s_k * u2_scale,
                    )
                    terms.append((1 if mlp_w[k] / wa > 0 else -1, k))

                # scalar-free signed pairwise tree, in place into the left
                # operand's row.  TT gets the 2x fp16 mode (vs 1x-only STT).
                # One level-1 pair goes to the otherwise idle GPSIMD engine.
                pool_pair = terms[4:6] if len(terms) == 6 else []
                if pool_pair:
                    terms = terms[:4]
                    (ta, ka), (tb, kb) = pool_pair
                    nc.gpsimd.tensor_tensor(
                        out=row(ka),
                        in0=row(ka),
                        in1=row(kb),
                        op=OP.add if ta == tb else OP.subtract,
                    )
                    pool_pair = (ta, ka)
                while len(terms) > 1:
                    nxt = []
                    for i in range(0, len(terms) - 1, 2):
                        ta, ka = terms[i]
                        tb, kb = terms[i + 1]
                        nc.vector.tensor_tensor(
                            out=row(ka),
                            in0=row(ka),
                            in1=row(kb),
                            op=OP.add if ta == tb else OP.subtract,
                        )
                        nxt.append((ta, ka))
                    if len(terms) % 2:
                        nxt.append(terms[-1])
                    terms = nxt
                if pool_pair:
                    ta, ka = terms[0]
                    tb, kb = pool_pair
                    nc.vector.tensor_tensor(
                        out=row(ka),
                        in0=row(ka),
                        in1=row(kb),
                        op=OP.add if ta == tb else OP.subtract,
                    )
                    terms = [(ta, ka)]

                ot = pool_o.tile([P, MSG], F32, tag="o")
                if terms:
                    tau, ka = terms[0]
                    nc.scalar.activation(
                        out=ot[:],
                        in_=row(ka),
                        func=AF.Relu,
                        bias=mbc[:],
                        scale=wa * tau,
                    )
                else:
                    nc.vector.memset(ot[:], max(mlp_b, 0.0))
                nc.sync.dma_start(out=out[n0 : n0 + P, :], in_=ot[:])
    _split_multi_waits(nc)
    return nc


def run(inputs, trace=False, **spmd_kwargs):
    """Build + run on 8 cores. Returns (full_output, BassKernelResults)."""
    msgs = np.asarray(inputs["messages"], dtype=np.float32)
    assert msgs.shape == (N_FULL, L, MSG), msgs.shape
    if not msgs.flags["C_CONTIGUOUS"]:
        msgs = np.ascontiguousarray(msgs)

    c1w = np.asarray(inputs["conv1_w"], dtype=np.float64)
    c2w = np.asarray(inputs["conv2_w"], dtype=np.float64)
    mlw = np.asarray(inputs["mlp_w"], dtype=np.float64)
    nc = build_program(
        float(c1w[0]),
        float(c1w[1]),
        float(np.asarray(inputs["conv1_b"], dtype=np.float64)),
        float(c2w[0]),
        float(c2w[1]),
        float(np.asarray(inputs["conv2_b"], dtype=np.float64)),
        [float(v) for v in mlw],
        float(np.asarray(inputs["mlp_b"], dtype=np.float64)),
    )

    in_maps = [
        {"x": msgs[i * N_LOCAL : (i + 1) * N_LOCAL]} for i in range(N_CORES)
    ]
    res = run_bass_kernel_spmd(
        nc, in_maps, core_ids=list(range(N_CORES)), trace=trace, **spmd_kwargs
    )
    full = np.concatenate([r["out"] for r in res.results], axis=0)
    return full, res


def kernel(**inputs) -> np.ndarray:
    return run(inputs, trace=False)[0]
